# revision 30
# baseline (speedup 1.0000x reference)
"""Trainium2 Bass kernel for an 8-expert top-2 MoE layer.

Strategy (expert-parallel + gate-adaptive fp8 precision ladder): the host
computes the (tiny) gating matmul + softmax + top-2 routing, gathers each
expert's assigned tokens SORTED BY GATE ASCENDING (padding in front), and
ships one expert per NeuronCore. All heavy compute runs as fp8e4m3
DoubleRow matmuls (0.5 cycles/row for K=256 vs bf16's 1.0 for K=128 —
4x PE throughput), with precision recovered via residual ("lo") fp8
correction terms:

  W ~= fp8(W*32) + fp8(W*32 - hi)     x ~= fp8(x*8) + fp8(x*8 - hi)

  L1 slab s (K=256): Whi@xhi + Wlo@xhi always; + Whi@xlo for the last
      Kx[s] (highest-gate) tokens.
  h eviction: h8 = relu(psum*2^-5 + 8*b1) -> fp8 directly (one ACT op);
      where h_lo is needed, a 3-op path (ACT->bf16, ACT copy->fp8,
      DVE subtract->fp8 residual).
  L2 slab s: W2hi@h_hi + W2lo@h_hi always; + W2hi@h_lo for the last
      Kh[s] tokens.
  y eviction: DVE (psum + 256*b2)*2^-8 -> fp16.

The per-(token,expert) quantization error is damped by that pair's gate
in the final combine, so low-gate tokens (the bulk; gates are flat-ish
~0.17-0.25) tolerate single-fp8 x/h operands while the few high-gate
tokens get full residual correction. Residual-term suffix boundaries
(Kx/Kh per slab, tuned in a host-side exact numerics simulator against
the 2e-2 absmax gate) make the ladder continuous: matmuls just cover
column sub-ranges of each tile, so no extra padding or segmenting.
Host-sim predicts 1.61e-2 absmax rel err (gate 2e-2); PE work is
~160k TT-cycles/core vs 232k for the previous bf16+partial-fp8 kernel.

All PSUM groups start and stop on full-width matmuls (suffix terms sit
in the middle) so partial-width accumulation is well-defined. Gate
multiply + top-2 combine stay on the host (exact fp32).
"""

import numpy as np

NUM_EXPERTS = 8
TOP_K = 2
D = 1024

# residual-term suffix boundaries as fractions of C (tuned in schedsim
# against the 2e-2 absmax gate): K*[s] = highest-gate token count getting
# that correction term for slab s. The W-side corrections cover almost
# everything (the lowest-gate ~15% of tokens tolerate raw W-noise); the
# x-lo/h-lo corrections only the high-gate head of the distribution.
FRAC_X = (0.2931, 0.2128, 0.1371, 0.0662)
FRAC_W = (0.8511, 0.8511, 0.8511, 0.8511)
FRAC_2 = (0.8511, 0.8511, 0.8511, 0.8511)
FRAC_H = (0.8511, 0.4965, 0.1371, 0.0662)

_prog_cache = {}


def _plan_tiles(max_load):
    """Token-tile sizes covering max_load exactly, ascending-gate order.

    Head tiles are 512 (one fp32 PSUM bank; wide enough that the L1
    strip groups don't outrun the w1 strip feed in the DMA-bound head).
    The remainder is split into two roughly equal tiles >= 128 at the
    (expensive, high-gate) tail so its L2/evictions overlap better.
    """
    C = max(int(max_load), 256)
    tiles = []
    rest = C
    while rest > 512:
        if rest - 512 >= 256 or rest - 512 == 0:
            tiles.append(512)
            rest -= 512
        else:
            a = rest // 2
            tiles.extend([rest - a, a])
            rest = 0
    if rest:
        tiles.append(rest)
    return C, tiles


def _bounds(C):
    Kx = [min(C, int(round(f * C))) for f in FRAC_X]
    Kw = [min(C, int(round(f * C))) for f in FRAC_W]
    K2 = [min(C, int(round(f * C))) for f in FRAC_2]
    Kh = [min(C, int(round(f * C))) for f in FRAC_H]
    return Kx, Kw, K2, Kh


def _build_program(C, tok_tiles, Kx, Kw, K2, Kh, n_warm=5, x0_split=False):
    """Per-core Bass program: one expert's 2-layer MLP over C tokens."""
    from contextlib import ExitStack

    import concourse.tile as tile
    from concourse import bacc, mybir

    f32 = mybir.dt.float32
    bf16 = mybir.dt.bfloat16
    f16 = mybir.dt.float16
    f8 = mybir.dt.float8e4
    DR = mybir.MatmulPerfMode.DoubleRow
    RELU = mybir.ActivationFunctionType.Relu
    COPY = mybir.ActivationFunctionType.Copy
    ADD = mybir.AluOpType.add
    MULT = mybir.AluOpType.mult
    SUB = mybir.AluOpType.subtract

    nc = bacc.Bacc("TRN2", target_bir_lowering=False, debug=False,
                   num_devices=NUM_EXPERTS)

    T = len(tok_tiles)
    tile_pos = [0]
    for TT in tok_tiles:
        tile_pos.append(tile_pos[-1] + TT)
    TT0 = tok_tiles[0]
    TTe = tok_tiles[-1]

    # host-packed layouts (ascending gate order, padding at the front):
    #   w1:  [8, 128, 2, 4, 2, 128]  w1[j, p, v, s, k, r] =
    #        q(32*W1[(2s+k)*128+p, j*128+r]) for v=0 (hi); residual v=1
    #   w2:  [8, 128, 2, 4, 2, 128]  same over W2 with j->o output strips
    #   x0:  [128, 4, 2, TT0]        head tile's xhi, own contiguous tensor
    #   xhi: [128, 4, 2, C]          xhi[p, s, k, c] = q(8*x_c[(2s+k)*128+p])
    #   xlo: [128, sum_s 2*Kx[s]]    per-slab suffix residuals, concatenated
    #   bb:  [128, 16]               [:, j] = 8*b1[j*128+p]; [:, 8+o] = 256*b2
    #   yT:  [128, 8, C] f16         yT[p, o, c] = y_c[o*128+p]
    #   yE:  [128, 8*TTe] f16        last tile's output (contiguous tail)
    w1_d = nc.dram_tensor("w1", [8, 128, 2, 4, 2, 128], f8,
                          kind="ExternalInput").ap()
    w2_d = nc.dram_tensor("w2", [8, 128, 2, 4, 2, 128], f8,
                          kind="ExternalInput").ap()
    x0_d = nc.dram_tensor("x0", [128, 4, 2, TT0], f8,
                          kind="ExternalInput").ap()
    xhi_d = nc.dram_tensor("xhi", [128, 4, 2, C], f8,
                           kind="ExternalInput").ap()
    # x residuals for the last Kxm token positions, all 4 slabs padded to
    # Kxm so slab slices are plain strided views (zeros where unused)
    Kxm = max(max(Kx), 1)
    xlo_d = nc.dram_tensor("xlo", [128, 4, 2, Kxm], f8,
                           kind="ExternalInput").ap()
    bb_d = nc.dram_tensor("bb", [128, 16], f32, kind="ExternalInput").ap()
    yT_d = nc.dram_tensor("yT", [128, 8, C], f16, kind="ExternalOutput").ap()
    yE_d = nc.dram_tensor("yE", [128, 8 * TTe], f16,
                          kind="ExternalOutput").ap()

    with tile.TileContext(nc) as tc, ExitStack() as ctx:
        wpool = ctx.enter_context(tc.tile_pool(name="w", bufs=1))
        cpool = ctx.enter_context(tc.tile_pool(name="const", bufs=1))
        xpool = ctx.enter_context(tc.tile_pool(name="x", bufs=1))
        hpool = ctx.enter_context(tc.tile_pool(name="h", bufs=2))
        bpool = ctx.enter_context(tc.tile_pool(name="hb", bufs=3))
        ypool = ctx.enter_context(tc.tile_pool(name="y", bufs=3))
        php = ctx.enter_context(tc.tile_pool(name="ph", bufs=4, space="PSUM"))
        pyp = ctx.enter_context(tc.tile_pool(name="py", bufs=4, space="PSUM"))

        # PE warm-up: on-chip zeros so dummy matmuls ride out the HAM clock
        # ramp while the DMA-bound head (w1 strip 0 + x0) lands.
        wz = cpool.tile([1, 640], bf16, tag="wz")
        nc.vector.memzero(wz[:])
        for _ in range(n_warm):
            warm = php.tile([128, 512], f32, tag="ph")
            nc.tensor.matmul(warm[:], wz[:, 0:128], wz[:, 128:640],
                             start=True, stop=True)

        # ---- DMA emission in consumption order ----
        # SP queue (strictly ordered): w1 strips, w2 strips, xlo, xhi
        # tiles 1..T-1, then per-tile y outputs as they are produced.
        # ACT queue: x0 (head tile) + biases, landing alongside w1 strip 0.
        w1_sb = []
        w1_0 = wpool.tile([128, 2, 4, 2, 128], f8, tag="w1_0")
        nc.sync.dma_start(w1_0[:], w1_d[0])
        w1_sb.append(w1_0)

        # x0 rides the SP queue right after w1 strip 0 (the serial DMA bus
        # delivers in dispatch order; the first L1 group needs both).
        x0_sb = xpool.tile([128, 4, 2, TT0], f8, tag="x0")
        bb_sb = cpool.tile([128, 16], f32, tag="bb")
        if x0_split:
            nc.sync.dma_start(x0_sb[:], x0_d[:])
            nc.scalar.dma_start(bb_sb[:], bb_d[:])
        else:
            nc.scalar.dma_start(x0_sb[:], x0_d[:])
            nc.scalar.dma_start(bb_sb[:], bb_d[:])
        b1_sb = bb_sb[:, 0:8]
        b2_sb = bb_sb[:, 8:16]

        # w2 strip 0 slots into the w1 stream early enough that L2(0) can
        # start right as L1(0) finishes; w1 strips 6-7 still outrun their
        # consumption by the L1 groups.
        w2_sb = [None] * 8
        for j in range(1, 8):
            if j == 6:
                w2_0 = wpool.tile([128, 2, 4, 2, 128], f8, tag="w2_0")
                nc.sync.dma_start(w2_0[:], w2_d[0])
                w2_sb[0] = w2_0
            w1_j = wpool.tile([128, 2, 4, 2, 128], f8, tag=f"w1_{j}")
            nc.sync.dma_start(w1_j[:], w1_d[j])
            w1_sb.append(w1_j)
        for o in range(1, 8):
            w2_o = wpool.tile([128, 2, 4, 2, 128], f8, tag=f"w2_{o}")
            nc.sync.dma_start(w2_o[:], w2_d[o])
            w2_sb[o] = w2_o

        # xhi tiles: tile 0 from x0; tiles 1.. from xhi_d slices, with
        # trailing sub-512 tiles grouped into one transfer (>=512B runs).
        # The xlo bundle slots in only before the first tile that needs
        # it (suffix terms start ~70% of the way through the schedule).
        x_sb = [None] * T
        x_base = [0] * T       # column offset of tile t inside its sb tile
        x_sb[0] = x0_sb
        xlo_sb = xpool.tile([128, 4, 2, Kxm], f8, tag="xlo")
        xlo_sent = False
        t = 1
        while t < T:
            if not xlo_sent and tile_pos[t + 1] > C - max(Kx):
                nc.sync.dma_start(xlo_sb[:], xlo_d[:])
                xlo_sent = True
            if tok_tiles[t] >= 512 or t == T - 1:
                xt = xpool.tile([128, 4, 2, tok_tiles[t]], f8, tag=f"x{t}")
                nc.sync.dma_start(
                    xt[:], xhi_d[:, :, :, tile_pos[t]:tile_pos[t + 1]])
                x_sb[t] = xt
                t += 1
            else:
                w = C - tile_pos[t]
                xt = xpool.tile([128, 4, 2, w], f8, tag=f"x{t}")
                nc.sync.dma_start(xt[:], xhi_d[:, :, :, tile_pos[t]:C])
                for u in range(t, T):
                    x_sb[u] = xt
                    x_base[u] = tile_pos[u] - tile_pos[t]
                t = T
                break
        if not xlo_sent:
            nc.sync.dma_start(xlo_sb[:], xlo_d[:])

        def emit_l1(t, h_dst, alt_pool=False):
            """Layer 1 for tile t -> h_hi (fp8, [128, 8, TT]) and
            h_lo (fp8, suffix columns only) in h_dst = (hhi, hlo)."""
            TT = tok_tiles[t]
            pos = tile_pos[t]
            hhi_t, hlo_t = h_dst
            xs = x_sb[t]
            xb = x_base[t]
            for j in range(8):
                pool, tag = ((pyp, "py") if alt_pool and j % 2 else
                             (php, "ph"))
                ph = pool.tile([128, 512], f32, tag=tag)
                # slab 0 hi first (full width, start=True)
                nc.tensor.matmul(ph[:, 0:TT], w1_sb[j][:, 0, 0],
                                 xs[:, 0, :, xb:xb + TT],
                                 start=True, stop=False, perf_mode=DR)
                for s in range(4):
                    if s > 0 and s < 3:
                        nc.tensor.matmul(ph[:, 0:TT], w1_sb[j][:, 0, s],
                                         xs[:, s, :, xb:xb + TT],
                                         start=False, stop=False,
                                         perf_mode=DR)
                    # W1 residual (suffix)
                    v0 = max(0, (C - Kw[s]) - pos)
                    if v0 < TT and Kw[s] > 0:
                        nc.tensor.matmul(ph[:, v0:TT], w1_sb[j][:, 1, s],
                                         xs[:, s, :, xb + v0:xb + TT],
                                         start=False, stop=False,
                                         perf_mode=DR)
                    # x residual (suffix of highest-gate tokens)
                    u0 = max(0, (C - Kx[s]) - pos)
                    if u0 < TT and Kx[s] > 0:
                        i0 = pos + u0 - (C - Kxm)
                        nc.tensor.matmul(ph[:, u0:TT], w1_sb[j][:, 0, s],
                                         xlo_sb[:, s, :,
                                                i0:i0 + (TT - u0)],
                                         start=False, stop=False,
                                         perf_mode=DR)
                # slab 3 hi last (full width, stop=True)
                nc.tensor.matmul(ph[:, 0:TT], w1_sb[j][:, 0, 3],
                                 xs[:, 3, :, xb:xb + TT],
                                 start=False, stop=True, perf_mode=DR)
                # eviction: h8 = relu(psum*2^-5 + 8*b1) (= 8*h). Tokens in
                # the h-lo suffix go through a bf16 intermediate (so the
                # fp8 residual can be formed); tokens before it evict
                # straight to fp8 — exactly matching the host simulator.
                hs = max(0, (C - Kh[j // 2]) - pos)
                if hs < TT:
                    if hs > 0:
                        nc.scalar.activation(hhi_t[:, j, 0:hs],
                                             ph[:, 0:hs], RELU,
                                             bias=b1_sb[:, j:j + 1],
                                             scale=2.0 ** -5)
                    hb = bpool.tile([128, TT], bf16, tag="hb")
                    nc.scalar.activation(hb[:, 0:TT - hs], ph[:, hs:TT],
                                         RELU, bias=b1_sb[:, j:j + 1],
                                         scale=2.0 ** -5)
                    nc.scalar.activation(hhi_t[:, j, hs:TT],
                                         hb[:, 0:TT - hs], COPY)
                    nc.vector.tensor_tensor(hlo_t[:, j, hs:TT],
                                            hb[:, 0:TT - hs],
                                            hhi_t[:, j, hs:TT], op=SUB)
                else:
                    nc.scalar.activation(hhi_t[:, j, 0:TT], ph[:, 0:TT],
                                         RELU, bias=b1_sb[:, j:j + 1],
                                         scale=2.0 ** -5)

        def emit_l2_group(t, h_src, o, py, c0, c1):
            """One L2 PSUM accumulation group: o-strip columns [c0, c1)."""
            TT = tok_tiles[t]
            pos = tile_pos[t]
            hhi_t, hlo_t = h_src
            nc.tensor.matmul(py[:, c0:c1], w2_sb[o][:, 0, 0],
                             hhi_t[:, 0:2, c0:c1],
                             start=True, stop=False, perf_mode=DR)
            for s in range(4):
                if s > 0 and s < 3:
                    nc.tensor.matmul(py[:, c0:c1], w2_sb[o][:, 0, s],
                                     hhi_t[:, 2 * s:2 * s + 2, c0:c1],
                                     start=False, stop=False, perf_mode=DR)
                v0 = max(c0, (C - K2[s]) - pos)
                if v0 < c1 and K2[s] > 0:
                    nc.tensor.matmul(py[:, v0:c1], w2_sb[o][:, 1, s],
                                     hhi_t[:, 2 * s:2 * s + 2, v0:c1],
                                     start=False, stop=False, perf_mode=DR)
                u0 = max(c0, (C - Kh[s]) - pos)
                if u0 < c1 and Kh[s] > 0:
                    nc.tensor.matmul(py[:, u0:c1], w2_sb[o][:, 0, s],
                                     hlo_t[:, 2 * s:2 * s + 2, u0:c1],
                                     start=False, stop=False, perf_mode=DR)
            nc.tensor.matmul(py[:, c0:c1], w2_sb[o][:, 0, 3],
                             hhi_t[:, 6:8, c0:c1],
                             start=False, stop=True, perf_mode=DR)

        def emit_l2(t, h_src, split_dma, tail=False):
            """Layer 2 for tile t from h_src = (hhi, hlo)."""
            TT = tok_tiles[t]
            pos = tile_pos[t]
            yt = ypool.tile([128, 8 * TT], f16, tag="y")
            for o in range(8):
                pool = pyp if (not tail or o % 2 == 0) else php
                py = pool.tile([128, 512], f32,
                               tag=("py" if pool is pyp else "ph"))
                # (two-piece final o-strip measured slower in TimelineSim:
                # the extra group's dispatch outweighs the shorter tail)
                two_piece = False
                cm = TT - 64 if two_piece else TT
                emit_l2_group(t, h_src, o, py, 0, cm)
                # evict: y = (psum + 256*b2) * 2^-8 -> fp16
                nc.vector.tensor_scalar(yt[:, o * TT:o * TT + cm],
                                        py[:, 0:cm], b2_sb[:, o:o + 1],
                                        2.0 ** -8, op0=ADD, op1=MULT)
                if two_piece:
                    nc.sync.dma_start(yE_d[:, 7 * TT:8 * TT - 64],
                                      yt[:, 7 * TT:8 * TT - 64])
                    emit_l2_group(t, h_src, o, py, cm, TT)
                    nc.vector.tensor_scalar(yt[:, o * TT + cm:(o + 1) * TT],
                                            py[:, cm:TT],
                                            b2_sb[:, o:o + 1],
                                            2.0 ** -8, op0=ADD, op1=MULT)
                if split_dma:
                    nc.sync.dma_start(yT_d[:, o, pos:pos + TT],
                                      yt[:, o * TT:(o + 1) * TT])
                if tail and o == 3:
                    nc.sync.dma_start(yE_d[:, 0:4 * TT], yt[:, 0:4 * TT])
                if tail and o == 6:
                    # leave only a tiny (1-strip) transfer on the critical
                    # tail after the final o=7 eviction
                    nc.sync.dma_start(yE_d[:, 4 * TT:7 * TT],
                                      yt[:, 4 * TT:7 * TT])
            if not split_dma:
                if tail:
                    nc.sync.dma_start(yE_d[:, 7 * TT:], yt[:, 7 * TT:])
                else:
                    nc.sync.dma_start(yT_d[:, :, pos:pos + TT], yt[:])

        # PE section order: L1(0), L2(0), ..., with the (expensive) last
        # tile's L1 hoisted before L2(T-2) so its evictions hide under
        # matmuls, and tile T-2's output leaving per-o-strip.
        h_tiles = []
        for t in range(T):
            TT = tok_tiles[t]
            hhi_t = hpool.tile([128, 8, TT], f8, tag="hhi")
            hlo_t = hpool.tile([128, 8, TT], f8, tag="hlo")
            h_tiles.append((hhi_t, hlo_t))
        for t in range(T):
            if t < T - 1:
                emit_l1(t, h_tiles[t])
                if t == T - 2:
                    emit_l1(T - 1, h_tiles[T - 1], alt_pool=True)
                emit_l2(t, h_tiles[t], split_dma=(t == T - 2))
            else:
                if T == 1:
                    emit_l1(t, h_tiles[t])
                emit_l2(t, h_tiles[t], split_dma=False, tail=True)

    nc.compile()
    return nc


def _route(x, Wg, bg):
    """Host gating: fp32 softmax + top-2, matching jax.lax.top_k semantics."""
    logits = x @ Wg + bg
    m = logits.max(axis=1, keepdims=True)
    e = np.exp(logits - m)
    gates = e / e.sum(axis=1, keepdims=True)
    # stable argsort on negated values = ties broken by lower index (jax)
    order = np.argsort(-gates, axis=1, kind="stable")[:, :TOP_K]
    return gates, order


def _make_in_maps(x, W1, b1, W2, b2, gates, order, tok_lists, C, Kx, Kh,
                  TT0):
    import ml_dtypes
    f8 = ml_dtypes.float8_e4m3fn

    def q8(v):
        return np.ascontiguousarray(v).astype(f8)

    def deq(v):
        return v.astype(np.float32)

    def pack_w(W):
        # [1024, 1024] -> [8, 128, 2, 4, 2, 128] hi/lo strips
        Ws = W * 32.0
        # Wt[o/j, p, s, k, r] = Ws[(2s+k)*128+p, j*128+r]
        Wt = Ws.reshape(4, 2, 128, 8, 128).transpose(3, 2, 0, 1, 4)
        hi = Wt.astype(f8)
        lo = (Wt - deq(hi)).astype(f8)
        return np.ascontiguousarray(
            np.stack([hi, lo], axis=2))  # [8, 128, 2, 4, 2, 128]

    Kxm = max(max(Kx), 1)
    in_maps = []
    for e in range(NUM_EXPERTS):
        toks = tok_lists[e]
        ne = len(toks)
        # ascending gate sort, padding (zeros) in FRONT
        g = gates[toks, e]
        asc = toks[np.argsort(g, kind="stable")]
        xs = np.zeros((C, D), dtype=np.float32)
        xs[C - ne:] = x[asc]
        # xhi[p, s, k, c] = q(8*xs[c, (2s+k)*128+p])
        x8 = (xs * 8.0).reshape(C, 4, 2, 128)      # [c, s, k, p]
        xhi = x8.astype(f8)                        # quantize
        xhi_t = np.ascontiguousarray(xhi.transpose(3, 1, 2, 0))
        xres = x8 - deq(xhi)                       # [c, s, k, p]
        xlo = np.zeros((128, 4, 2, Kxm), dtype=f8)
        for s in range(4):
            k = Kx[s]
            if k:
                # [p, 2, c] from residual rows of the last k tokens
                blk = xres[C - k:, s].transpose(2, 1, 0)
                xlo[:, s, :, Kxm - k:] = q8(blk)
        bb = np.concatenate([8.0 * b1[e].reshape(8, 128).T,
                             256.0 * b2[e].reshape(8, 128).T], axis=1)
        in_maps.append({
            "w1": pack_w(W1[e]),
            "w2": pack_w(W2[e]),
            "x0": np.ascontiguousarray(xhi_t[:, :, :, 0:TT0]),
            "xhi": xhi_t,
            "xlo": xlo,
            "bb": np.ascontiguousarray(bb.astype(np.float32)),
        })
    return in_maps, [np.argsort(gates[tok_lists[e], e], kind="stable")
                     for e in range(NUM_EXPERTS)]


def kernel(x, W1, b1, W2, b2, Wg, bg):
    from concourse import bass_utils

    x = np.ascontiguousarray(np.asarray(x, dtype=np.float32))
    W1 = np.asarray(W1, dtype=np.float32)
    b1 = np.asarray(b1, dtype=np.float32)
    W2 = np.asarray(W2, dtype=np.float32)
    b2 = np.asarray(b2, dtype=np.float32)
    Wg = np.asarray(Wg, dtype=np.float32)
    bg = np.asarray(bg, dtype=np.float32)
    n = x.shape[0]

    gates, order = _route(x, Wg, bg)
    tok_lists = [np.where((order == e).any(axis=1))[0]
                 for e in range(NUM_EXPERTS)]
    max_load = max(len(t) for t in tok_lists)
    C, tok_tiles = _plan_tiles(max_load)
    Kx, Kw, K2, Kh = _bounds(C)

    key = (C, tuple(tok_tiles), tuple(Kx), tuple(Kw), tuple(K2), tuple(Kh))
    if key not in _prog_cache:
        _prog_cache[key] = _build_program(C, tok_tiles, Kx, Kw, K2, Kh)
    nc = _prog_cache[key]

    in_maps, asc_orders = _make_in_maps(
        x, W1, b1, W2, b2, gates, order, tok_lists, C, Kx, Kh, tok_tiles[0])
    res = bass_utils.run_bass_kernel_spmd(nc, in_maps,
                                          list(range(NUM_EXPERTS)))

    TTe = tok_tiles[-1]
    out = np.zeros((n, D), dtype=np.float32)
    for e in range(NUM_EXPERTS):
        toks = tok_lists[e]
        ne = len(toks)
        yT = np.asarray(res.results[e]["yT"], dtype=np.float32)
        yE = np.asarray(res.results[e]["yE"], dtype=np.float32)
        yT[:, :, C - TTe:] = yE.reshape(128, 8, TTe)
        # yT[p, o, c] -> y[c, o*128+p]; positions C-ne.. hold the sorted toks
        y = yT[:, :, C - ne:].transpose(2, 1, 0).reshape(ne, D)
        asc = toks[asc_orders[e]]
        out[asc] += gates[asc, e][:, None] * y
    return out


# revision 36
# speedup vs baseline: 1.0013x; 1.0013x over previous
"""Trainium2 Bass kernel for an 8-expert top-2 MoE layer.

Strategy (expert-parallel + gate-adaptive fp8 precision ladder): the host
computes the (tiny) gating matmul + softmax + top-2 routing, gathers each
expert's assigned tokens SORTED BY GATE ASCENDING (padding in front), and
ships one expert per NeuronCore. All heavy compute runs as fp8e4m3
DoubleRow matmuls (0.5 cycles/output-column for a K=256 contraction vs
bf16's 1.0 for K=128 — 4x PE throughput), with precision recovered via
residual ("lo") fp8 correction terms:

  W ~= fp8(W*32) + fp8(W*32 - hi)     x ~= fp8(x*8) + fp8(x*8 - hi)

  L1 slab s (K=256): Whi@xhi always; + Wlo@xhi for the last Kw[s]
      (highest-gate) tokens; + Whi@xlo for the last Kx[s].
  h eviction: h8 = relu(psum*2^-5 + 8*b1) -> fp8 in one ACT op; tokens
      in the h-lo suffix go through a 3-op path (ACT->bf16, ACT
      copy->fp8, DVE subtract->fp8 residual) matching the host
      simulator bit-for-bit.
  L2 slab s: W2hi@h_hi always; + W2lo@h_hi for the last K2[s] tokens;
      + W2hi@h_lo for the last Kh[s].
  y eviction: DVE (psum + 256*b2)*2^-8 -> fp16.

The per-(token,expert) quantization error is damped by that pair's gate
in the final combine, so low-gate tokens tolerate single-fp8 x/h (and,
for the bottom ~15%, single-fp8 W) operands while high-gate tokens get
full residual correction. Suffix boundaries per slab were tuned in a
host-side exact numerics simulator (schedsim.py) against the 2e-2
absmax gate; since all residual terms are additive, matmuls just cover
column sub-ranges of each tile — the ladder is continuous with no extra
padding or segmenting. Every PSUM group starts and stops on full-width
matmuls (suffix terms sit in the middle) so partial-width accumulation
is well-defined. Measured on the graded inputs: rel err 1.722e-2,
TimelineSim 72394 ns/core (previous bf16+partial-fp8 kernel: 1.444e-2,
111189 ns).

Schedule notes (from TimelineSim traces):
 - DMA bus is serial; SP-queue order = consumption order: w1 strips
   (w2 strip 0 interleaved before w1 s6), w2 strips, xhi tile
   prefetches with the xlo bundle deferred until just before the first
   suffix tile, then per-tile y outputs. x0 + biases ride the ACT
   queue. The head is DMA-bound (~5.1us to land w1 strip 0 + the full
   x0 tile; then L1(0) is strip-feed-bound at 728ns/strip).
 - The (expensive, high-gate) last tile's L1 is hoisted before tile
   T-2's L2; tile T-2's output leaves per-o-strip; the last tile's
   output leaves in three pieces (o<=3, o<=6, o=7) so only one strip
   trails the final eviction.
Gate multiply + top-2 combine stay on the host (exact fp32).
"""

import numpy as np

NUM_EXPERTS = 8
TOP_K = 2
D = 1024

# residual-term suffix boundaries as fractions of C (tuned in schedsim
# against the 2e-2 absmax gate): K*[s] = highest-gate token count getting
# that correction term for slab s. The W-side corrections cover almost
# everything (the lowest-gate ~15% of tokens tolerate raw W-noise); the
# x-lo/h-lo corrections only the high-gate head of the distribution.
FRAC_X = (0.2931, 0.2128, 0.1371, 0.0662)
FRAC_W = (0.8511, 0.8511, 0.8511, 0.8511)
FRAC_2 = (0.8511, 0.8511, 0.8511, 0.8511)
FRAC_H = (0.8511, 0.4965, 0.1371, 0.0662)

_prog_cache = {}


def _plan_tiles(max_load):
    """Token-tile sizes covering max_load exactly, ascending-gate order.

    Head tiles are 512 (one fp32 PSUM bank; wide enough that the L1
    strip groups don't outrun the w1 strip feed in the DMA-bound head).
    The remainder is split into two roughly equal tiles >= 128 at the
    (expensive, high-gate) tail so its L2/evictions overlap better.
    """
    C = max(int(max_load), 256)
    tiles = []
    rest = C
    while rest > 512:
        if rest - 512 >= 256 or rest - 512 == 0:
            tiles.append(512)
            rest -= 512
        else:
            a = rest // 2
            tiles.extend([rest - a, a])
            rest = 0
    if rest:
        tiles.append(rest)
    return C, tiles


def _bounds(C):
    Kx = [min(C, int(round(f * C))) for f in FRAC_X]
    Kw = [min(C, int(round(f * C))) for f in FRAC_W]
    K2 = [min(C, int(round(f * C))) for f in FRAC_2]
    Kh = [min(C, int(round(f * C))) for f in FRAC_H]
    return Kx, Kw, K2, Kh


def _build_program(C, tok_tiles, Kx, Kw, K2, Kh, n_warm=5):
    """Per-core Bass program: one expert's 2-layer MLP over C tokens."""
    from contextlib import ExitStack

    import concourse.tile as tile
    from concourse import bacc, mybir

    f32 = mybir.dt.float32
    bf16 = mybir.dt.bfloat16
    f16 = mybir.dt.float16
    f8 = mybir.dt.float8e4
    DR = mybir.MatmulPerfMode.DoubleRow
    RELU = mybir.ActivationFunctionType.Relu
    COPY = mybir.ActivationFunctionType.Copy
    ADD = mybir.AluOpType.add
    MULT = mybir.AluOpType.mult
    SUB = mybir.AluOpType.subtract

    nc = bacc.Bacc("TRN2", target_bir_lowering=False, debug=False,
                   num_devices=NUM_EXPERTS)

    T = len(tok_tiles)
    tile_pos = [0]
    for TT in tok_tiles:
        tile_pos.append(tile_pos[-1] + TT)
    TT0 = tok_tiles[0]
    TTe = tok_tiles[-1]

    # host-packed layouts (ascending gate order, padding at the front):
    #   w1:  [8, 128, 2, 4, 2, 128]  w1[j, p, v, s, k, r] =
    #        q(32*W1[(2s+k)*128+p, j*128+r]) for v=0 (hi); residual v=1
    #   w2:  [8, 128, 2, 4, 2, 128]  same over W2 with j->o output strips
    #   x0:  [128, 4, 2, TT0]        head tile's xhi, own contiguous tensor
    #   xhi: [128, 4, 2, C]          xhi[p, s, k, c] = q(8*x_c[(2s+k)*128+p])
    #   xlo: [128, sum_s 2*Kx[s]]    per-slab suffix residuals, concatenated
    #   bb:  [128, 16]               [:, j] = 8*b1[j*128+p]; [:, 8+o] = 256*b2
    #   yT:  [128, 8, C] f16         yT[p, o, c] = y_c[o*128+p]
    #   yE:  [128, 8*TTe] f16        last tile's output (contiguous tail)
    w1_d = nc.dram_tensor("w1", [8, 128, 2, 4, 2, 128], f8,
                          kind="ExternalInput").ap()
    w2_d = nc.dram_tensor("w2", [8, 128, 2, 4, 2, 128], f8,
                          kind="ExternalInput").ap()
    x0_d = nc.dram_tensor("x0", [128, 4, 2, TT0], f8,
                          kind="ExternalInput").ap()
    xhi_d = nc.dram_tensor("xhi", [128, 4, 2, C], f8,
                           kind="ExternalInput").ap()
    # x residuals for the last Kxm token positions, all 4 slabs padded to
    # Kxm so slab slices are plain strided views (zeros where unused)
    Kxm = max(max(Kx), 1)
    xlo_d = nc.dram_tensor("xlo", [128, 4, 2, Kxm], f8,
                           kind="ExternalInput").ap()
    bb_d = nc.dram_tensor("bb", [128, 16], f32, kind="ExternalInput").ap()
    yT_d = nc.dram_tensor("yT", [128, 8, C], f16, kind="ExternalOutput").ap()
    yE_d = nc.dram_tensor("yE", [128, 8 * TTe], f16,
                          kind="ExternalOutput").ap()

    with tile.TileContext(nc) as tc, ExitStack() as ctx:
        wpool = ctx.enter_context(tc.tile_pool(name="w", bufs=1))
        cpool = ctx.enter_context(tc.tile_pool(name="const", bufs=1))
        xpool = ctx.enter_context(tc.tile_pool(name="x", bufs=1))
        hpool = ctx.enter_context(tc.tile_pool(name="h", bufs=2))
        bpool = ctx.enter_context(tc.tile_pool(name="hb", bufs=3))
        ypool = ctx.enter_context(tc.tile_pool(name="y", bufs=3))
        php = ctx.enter_context(tc.tile_pool(name="ph", bufs=4, space="PSUM"))
        pyp = ctx.enter_context(tc.tile_pool(name="py", bufs=4, space="PSUM"))

        # PE warm-up: on-chip zeros so dummy matmuls ride out the HAM clock
        # ramp while the DMA-bound head (w1 strip 0 + x0) lands.
        wz = cpool.tile([1, 640], bf16, tag="wz")
        nc.vector.memzero(wz[:])
        for _ in range(n_warm):
            warm = php.tile([128, 512], f32, tag="ph")
            nc.tensor.matmul(warm[:], wz[:, 0:128], wz[:, 128:640],
                             start=True, stop=True)

        # ---- DMA emission in consumption order ----
        # SP queue (strictly ordered): w1 strips, w2 strips, xlo, xhi
        # tiles 1..T-1, then per-tile y outputs as they are produced.
        # ACT queue: x0 (head tile) + biases, landing alongside w1 strip 0.
        w1_sb = []
        w1_0 = wpool.tile([128, 2, 4, 2, 128], f8, tag="w1_0")
        nc.sync.dma_start(w1_0[:], w1_d[0])
        w1_sb.append(w1_0)

        # x0 + biases ride the ACT queue, landing alongside w1 strip 0
        # (per-slab x0 splits and SP placement both measured slower: the
        # extra HWDGE dispatches outweigh the earlier first slab).
        x0_sb = xpool.tile([128, 4, 2, TT0], f8, tag="x0")
        bb_sb = cpool.tile([128, 16], f32, tag="bb")
        nc.scalar.dma_start(x0_sb[:], x0_d[:])
        nc.scalar.dma_start(bb_sb[:], bb_d[:])
        b1_sb = bb_sb[:, 0:8]
        b2_sb = bb_sb[:, 8:16]

        # w2 strip 0 slots into the w1 stream early enough that L2(0) can
        # start right as L1(0) finishes; w1 strips 6-7 still outrun their
        # consumption by the L1 groups.
        w2_sb = [None] * 8
        for j in range(1, 8):
            if j == 6:
                w2_0 = wpool.tile([128, 2, 4, 2, 128], f8, tag="w2_0")
                nc.sync.dma_start(w2_0[:], w2_d[0])
                w2_sb[0] = w2_0
            w1_j = wpool.tile([128, 2, 4, 2, 128], f8, tag=f"w1_{j}")
            nc.sync.dma_start(w1_j[:], w1_d[j])
            w1_sb.append(w1_j)
        x1_early = None
        for o in range(1, 8):
            if o == 7 and T > 1 and tok_tiles[1] >= 512:
                # x1 slots in just before the last w2 strip: it lands right
                # as tile 1's L1 starts, and w2 strip 7 still beats L2(0)'s
                # final o-group
                x1_early = xpool.tile([128, 4, 2, tok_tiles[1]], f8,
                                      tag="x1")
                nc.sync.dma_start(
                    x1_early[:], xhi_d[:, :, :, tile_pos[1]:tile_pos[2]])
            w2_o = wpool.tile([128, 2, 4, 2, 128], f8, tag=f"w2_{o}")
            nc.sync.dma_start(w2_o[:], w2_d[o])
            w2_sb[o] = w2_o

        # xhi tiles: tile 0 from x0; tiles 1.. from xhi_d slices, with
        # trailing sub-512 tiles grouped into one transfer (>=512B runs).
        # The xlo bundle slots in only before the first tile that needs
        # it (suffix terms start ~70% of the way through the schedule).
        x_sb = [None] * T
        x_base = [0] * T       # column offset of tile t inside its sb tile
        x_sb[0] = x0_sb
        xlo_sb = xpool.tile([128, 4, 2, Kxm], f8, tag="xlo")
        xlo_sent = False
        t = 1
        if x1_early is not None:
            x_sb[1] = x1_early
            t = 2
        while t < T:
            if not xlo_sent and tile_pos[t + 1] > C - max(Kx):
                nc.sync.dma_start(xlo_sb[:], xlo_d[:])
                xlo_sent = True
            if tok_tiles[t] >= 512 or t == T - 1:
                xt = xpool.tile([128, 4, 2, tok_tiles[t]], f8, tag=f"x{t}")
                nc.sync.dma_start(
                    xt[:], xhi_d[:, :, :, tile_pos[t]:tile_pos[t + 1]])
                x_sb[t] = xt
                t += 1
            else:
                w = C - tile_pos[t]
                xt = xpool.tile([128, 4, 2, w], f8, tag=f"x{t}")
                nc.sync.dma_start(xt[:], xhi_d[:, :, :, tile_pos[t]:C])
                for u in range(t, T):
                    x_sb[u] = xt
                    x_base[u] = tile_pos[u] - tile_pos[t]
                t = T
                break
        if not xlo_sent:
            nc.sync.dma_start(xlo_sb[:], xlo_d[:])

        def emit_l1(t, h_dst, alt_pool=False):
            """Layer 1 for tile t -> h_hi (fp8, [128, 8, TT]) and
            h_lo (fp8, suffix columns only) in h_dst = (hhi, hlo)."""
            TT = tok_tiles[t]
            pos = tile_pos[t]
            hhi_t, hlo_t = h_dst
            xs = x_sb[t]
            xb = x_base[t]
            for j in range(8):
                pool, tag = ((pyp, "py") if alt_pool and j % 2 else
                             (php, "ph"))
                ph = pool.tile([128, 512], f32, tag=tag)
                # slab 0 hi first (full width, start=True)
                nc.tensor.matmul(ph[:, 0:TT], w1_sb[j][:, 0, 0],
                                 xs[:, 0, :, xb:xb + TT],
                                 start=True, stop=False, perf_mode=DR)
                for s in range(4):
                    if s > 0 and s < 3:
                        nc.tensor.matmul(ph[:, 0:TT], w1_sb[j][:, 0, s],
                                         xs[:, s, :, xb:xb + TT],
                                         start=False, stop=False,
                                         perf_mode=DR)
                    # W1 residual (suffix)
                    v0 = max(0, (C - Kw[s]) - pos)
                    if v0 < TT and Kw[s] > 0:
                        nc.tensor.matmul(ph[:, v0:TT], w1_sb[j][:, 1, s],
                                         xs[:, s, :, xb + v0:xb + TT],
                                         start=False, stop=False,
                                         perf_mode=DR)
                    # x residual (suffix of highest-gate tokens)
                    u0 = max(0, (C - Kx[s]) - pos)
                    if u0 < TT and Kx[s] > 0:
                        i0 = pos + u0 - (C - Kxm)
                        nc.tensor.matmul(ph[:, u0:TT], w1_sb[j][:, 0, s],
                                         xlo_sb[:, s, :,
                                                i0:i0 + (TT - u0)],
                                         start=False, stop=False,
                                         perf_mode=DR)
                # slab 3 hi last (full width, stop=True)
                nc.tensor.matmul(ph[:, 0:TT], w1_sb[j][:, 0, 3],
                                 xs[:, 3, :, xb:xb + TT],
                                 start=False, stop=True, perf_mode=DR)
                # eviction: h8 = relu(psum*2^-5 + 8*b1) (= 8*h). Tokens in
                # the h-lo suffix go through a bf16 intermediate (so the
                # fp8 residual can be formed); tokens before it evict
                # straight to fp8 — exactly matching the host simulator.
                hs = max(0, (C - Kh[j // 2]) - pos)
                if hs < TT:
                    if hs > 0:
                        nc.scalar.activation(hhi_t[:, j, 0:hs],
                                             ph[:, 0:hs], RELU,
                                             bias=b1_sb[:, j:j + 1],
                                             scale=2.0 ** -5)
                    hb = bpool.tile([128, TT], bf16, tag="hb")
                    nc.scalar.activation(hb[:, 0:TT - hs], ph[:, hs:TT],
                                         RELU, bias=b1_sb[:, j:j + 1],
                                         scale=2.0 ** -5)
                    nc.scalar.activation(hhi_t[:, j, hs:TT],
                                         hb[:, 0:TT - hs], COPY)
                    nc.vector.tensor_tensor(hlo_t[:, j, hs:TT],
                                            hb[:, 0:TT - hs],
                                            hhi_t[:, j, hs:TT], op=SUB)
                else:
                    nc.scalar.activation(hhi_t[:, j, 0:TT], ph[:, 0:TT],
                                         RELU, bias=b1_sb[:, j:j + 1],
                                         scale=2.0 ** -5)

        def emit_l2_group(t, h_src, o, py, c0, c1):
            """One L2 PSUM accumulation group: o-strip columns [c0, c1)."""
            TT = tok_tiles[t]
            pos = tile_pos[t]
            hhi_t, hlo_t = h_src
            nc.tensor.matmul(py[:, c0:c1], w2_sb[o][:, 0, 0],
                             hhi_t[:, 0:2, c0:c1],
                             start=True, stop=False, perf_mode=DR)
            for s in range(4):
                if s > 0 and s < 3:
                    nc.tensor.matmul(py[:, c0:c1], w2_sb[o][:, 0, s],
                                     hhi_t[:, 2 * s:2 * s + 2, c0:c1],
                                     start=False, stop=False, perf_mode=DR)
                v0 = max(c0, (C - K2[s]) - pos)
                if v0 < c1 and K2[s] > 0:
                    nc.tensor.matmul(py[:, v0:c1], w2_sb[o][:, 1, s],
                                     hhi_t[:, 2 * s:2 * s + 2, v0:c1],
                                     start=False, stop=False, perf_mode=DR)
                u0 = max(c0, (C - Kh[s]) - pos)
                if u0 < c1 and Kh[s] > 0:
                    nc.tensor.matmul(py[:, u0:c1], w2_sb[o][:, 0, s],
                                     hlo_t[:, 2 * s:2 * s + 2, u0:c1],
                                     start=False, stop=False, perf_mode=DR)
            nc.tensor.matmul(py[:, c0:c1], w2_sb[o][:, 0, 3],
                             hhi_t[:, 6:8, c0:c1],
                             start=False, stop=True, perf_mode=DR)

        def emit_l2(t, h_src, split_dma, tail=False):
            """Layer 2 for tile t from h_src = (hhi, hlo)."""
            TT = tok_tiles[t]
            pos = tile_pos[t]
            yt = ypool.tile([128, 8 * TT], f16, tag="y")
            for o in range(8):
                pool = pyp if (not tail or o % 2 == 0) else php
                py = pool.tile([128, 512], f32,
                               tag=("py" if pool is pyp else "ph"))
                emit_l2_group(t, h_src, o, py, 0, TT)
                # evict: y = (psum + 256*b2) * 2^-8 -> fp16
                nc.vector.tensor_scalar(yt[:, o * TT:(o + 1) * TT],
                                        py[:, 0:TT], b2_sb[:, o:o + 1],
                                        2.0 ** -8, op0=ADD, op1=MULT)
                if split_dma:
                    nc.sync.dma_start(yT_d[:, o, pos:pos + TT],
                                      yt[:, o * TT:(o + 1) * TT])
                if tail and o == 3:
                    nc.sync.dma_start(yE_d[:, 0:4 * TT], yt[:, 0:4 * TT])
                if tail and o == 6:
                    # leave only a tiny (1-strip) transfer on the critical
                    # tail after the final o=7 eviction
                    nc.sync.dma_start(yE_d[:, 4 * TT:7 * TT],
                                      yt[:, 4 * TT:7 * TT])
            if not split_dma:
                if tail:
                    nc.sync.dma_start(yE_d[:, 7 * TT:], yt[:, 7 * TT:])
                else:
                    nc.sync.dma_start(yT_d[:, :, pos:pos + TT], yt[:])

        # PE section order: L1(0), L2(0), ..., with the (expensive) last
        # tile's L1 hoisted before L2(T-2) so its evictions hide under
        # matmuls, and tile T-2's output leaving per-o-strip.
        h_tiles = []
        for t in range(T):
            TT = tok_tiles[t]
            hhi_t = hpool.tile([128, 8, TT], f8, tag="hhi")
            hlo_t = hpool.tile([128, 8, TT], f8, tag="hlo")
            h_tiles.append((hhi_t, hlo_t))
        for t in range(T):
            if t < T - 1:
                emit_l1(t, h_tiles[t])
                if t == T - 2:
                    emit_l1(T - 1, h_tiles[T - 1], alt_pool=True)
                emit_l2(t, h_tiles[t], split_dma=(t == T - 2))
            else:
                if T == 1:
                    emit_l1(t, h_tiles[t])
                emit_l2(t, h_tiles[t], split_dma=False, tail=True)

    nc.compile()
    return nc


def _route(x, Wg, bg):
    """Host gating: fp32 softmax + top-2, matching jax.lax.top_k semantics."""
    logits = x @ Wg + bg
    m = logits.max(axis=1, keepdims=True)
    e = np.exp(logits - m)
    gates = e / e.sum(axis=1, keepdims=True)
    # stable argsort on negated values = ties broken by lower index (jax)
    order = np.argsort(-gates, axis=1, kind="stable")[:, :TOP_K]
    return gates, order


def _make_in_maps(x, W1, b1, W2, b2, gates, order, tok_lists, C, Kx, Kh,
                  TT0):
    import ml_dtypes
    f8 = ml_dtypes.float8_e4m3fn

    def q8(v):
        return np.ascontiguousarray(v).astype(f8)

    def deq(v):
        return v.astype(np.float32)

    def pack_w(W):
        # [1024, 1024] -> [8, 128, 2, 4, 2, 128] hi/lo strips
        Ws = W * 32.0
        # Wt[o/j, p, s, k, r] = Ws[(2s+k)*128+p, j*128+r]
        Wt = Ws.reshape(4, 2, 128, 8, 128).transpose(3, 2, 0, 1, 4)
        hi = Wt.astype(f8)
        lo = (Wt - deq(hi)).astype(f8)
        return np.ascontiguousarray(
            np.stack([hi, lo], axis=2))  # [8, 128, 2, 4, 2, 128]

    Kxm = max(max(Kx), 1)
    in_maps = []
    for e in range(NUM_EXPERTS):
        toks = tok_lists[e]
        ne = len(toks)
        # ascending gate sort, padding (zeros) in FRONT
        g = gates[toks, e]
        asc = toks[np.argsort(g, kind="stable")]
        xs = np.zeros((C, D), dtype=np.float32)
        xs[C - ne:] = x[asc]
        # xhi[p, s, k, c] = q(8*xs[c, (2s+k)*128+p])
        x8 = (xs * 8.0).reshape(C, 4, 2, 128)      # [c, s, k, p]
        xhi = x8.astype(f8)                        # quantize
        xhi_t = np.ascontiguousarray(xhi.transpose(3, 1, 2, 0))
        xres = x8 - deq(xhi)                       # [c, s, k, p]
        xlo = np.zeros((128, 4, 2, Kxm), dtype=f8)
        for s in range(4):
            k = Kx[s]
            if k:
                # [p, 2, c] from residual rows of the last k tokens
                blk = xres[C - k:, s].transpose(2, 1, 0)
                xlo[:, s, :, Kxm - k:] = q8(blk)
        bb = np.concatenate([8.0 * b1[e].reshape(8, 128).T,
                             256.0 * b2[e].reshape(8, 128).T], axis=1)
        in_maps.append({
            "w1": pack_w(W1[e]),
            "w2": pack_w(W2[e]),
            "x0": np.ascontiguousarray(xhi_t[:, :, :, 0:TT0]),
            "xhi": xhi_t,
            "xlo": xlo,
            "bb": np.ascontiguousarray(bb.astype(np.float32)),
        })
    return in_maps, [np.argsort(gates[tok_lists[e], e], kind="stable")
                     for e in range(NUM_EXPERTS)]


def kernel(x, W1, b1, W2, b2, Wg, bg):
    from concourse import bass_utils

    x = np.ascontiguousarray(np.asarray(x, dtype=np.float32))
    W1 = np.asarray(W1, dtype=np.float32)
    b1 = np.asarray(b1, dtype=np.float32)
    W2 = np.asarray(W2, dtype=np.float32)
    b2 = np.asarray(b2, dtype=np.float32)
    Wg = np.asarray(Wg, dtype=np.float32)
    bg = np.asarray(bg, dtype=np.float32)
    n = x.shape[0]

    gates, order = _route(x, Wg, bg)
    tok_lists = [np.where((order == e).any(axis=1))[0]
                 for e in range(NUM_EXPERTS)]
    max_load = max(len(t) for t in tok_lists)
    C, tok_tiles = _plan_tiles(max_load)
    Kx, Kw, K2, Kh = _bounds(C)

    key = (C, tuple(tok_tiles), tuple(Kx), tuple(Kw), tuple(K2), tuple(Kh))
    if key not in _prog_cache:
        _prog_cache[key] = _build_program(C, tok_tiles, Kx, Kw, K2, Kh)
    nc = _prog_cache[key]

    in_maps, asc_orders = _make_in_maps(
        x, W1, b1, W2, b2, gates, order, tok_lists, C, Kx, Kh, tok_tiles[0])
    res = bass_utils.run_bass_kernel_spmd(nc, in_maps,
                                          list(range(NUM_EXPERTS)))

    TTe = tok_tiles[-1]
    out = np.zeros((n, D), dtype=np.float32)
    for e in range(NUM_EXPERTS):
        toks = tok_lists[e]
        ne = len(toks)
        yT = np.asarray(res.results[e]["yT"], dtype=np.float32)
        yE = np.asarray(res.results[e]["yE"], dtype=np.float32)
        yT[:, :, C - TTe:] = yE.reshape(128, 8, TTe)
        # yT[p, o, c] -> y[c, o*128+p]; positions C-ne.. hold the sorted toks
        y = yT[:, :, C - ne:].transpose(2, 1, 0).reshape(ne, D)
        asc = toks[asc_orders[e]]
        out[asc] += gates[asc, e][:, None] * y
    return out


# revision 51
# speedup vs baseline: 1.0096x; 1.0083x over previous
"""Trainium2 Bass kernel for an 8-expert top-2 MoE layer.

Strategy (expert-parallel + gate-adaptive fp8 precision ladder): the host
computes the (tiny) gating matmul + softmax + top-2 routing, gathers each
expert's assigned tokens SORTED BY GATE ASCENDING (padding in front), and
ships one expert per NeuronCore. All heavy compute runs as fp8e4m3
DoubleRow matmuls (0.5 cycles/output-column for a K=256 contraction vs
bf16's 1.0 for K=128 — 4x PE throughput), with precision recovered via
residual ("lo") fp8 correction terms:

  W ~= fp8(W*32) + fp8(W*32 - hi)     x ~= fp8(x*8) + fp8(x*8 - hi)

  L1 slab s (K=256): Whi@xhi always; + Wlo@xhi for the last Kw[s]
      (highest-gate) tokens; + Whi@xlo for the last Kx[s].
  h eviction: h8 = relu(psum*2^-5 + 8*b1) -> fp8 in one ACT op; tokens
      in the h-lo suffix go through a 3-op path (ACT->bf16, ACT
      copy->fp8, DVE subtract->fp8 residual) matching the host
      simulator bit-for-bit.
  L2 slab s: W2hi@h_hi always; + W2lo@h_hi for the last K2[s] tokens;
      + W2hi@h_lo for the last Kh[s].
  y eviction: DVE (psum + 256*b2)*2^-8 -> fp16.

The per-(token,expert) quantization error is damped by that pair's gate
in the final combine, so low-gate tokens tolerate single-fp8 x/h (and,
for the bottom ~15%, single-fp8 W) operands while high-gate tokens get
full residual correction. Suffix boundaries per slab were tuned in a
host-side exact numerics simulator (schedsim.py) against the 2e-2
absmax gate; since all residual terms are additive, matmuls just cover
column sub-ranges of each tile — the ladder is continuous with no extra
padding or segmenting. Every PSUM group starts and stops on full-width
matmuls (suffix terms sit in the middle) so partial-width accumulation
is well-defined. Measured on the graded inputs: rel err 1.722e-2,
TimelineSim 72394 ns/core (previous bf16+partial-fp8 kernel: 1.444e-2,
111189 ns).

Schedule notes (from TimelineSim traces):
 - DMA bus is serial; SP-queue order = consumption order: w1 strips
   (w2 strip 0 interleaved before w1 s6), w2 strips, xhi tile
   prefetches with the xlo bundle deferred until just before the first
   suffix tile, then per-tile y outputs. x0 + biases ride the ACT
   queue. The head is DMA-bound (~5.1us to land w1 strip 0 + the full
   x0 tile; then L1(0) is strip-feed-bound at 728ns/strip).
 - The (expensive, high-gate) last tile's L1 is hoisted before tile
   T-2's L2; tile T-2's output leaves per-o-strip; the last tile's
   output leaves in three pieces (o<=3, o<=6, o=7) so only one strip
   trails the final eviction.
Gate multiply + top-2 combine stay on the host (exact fp32).
"""

import numpy as np

NUM_EXPERTS = 8
TOP_K = 2
D = 1024

# residual-term suffix boundaries as fractions of C (tuned in schedsim
# against the 2e-2 absmax gate): K*[s] = highest-gate token count getting
# that correction term for slab s. The W-side corrections cover almost
# everything (the lowest-gate ~15% of tokens tolerate raw W-noise); the
# x-lo/h-lo corrections only the high-gate head of the distribution.
FRAC_X = (0.2931, 0.2128, 0.1371, 0.0662)
FRAC_W = (0.8511, 0.8511, 0.8511, 0.8511)
FRAC_2 = (0.8511, 0.8511, 0.8511, 0.8511)
FRAC_H = (0.8511, 0.4965, 0.1371, 0.0662)

_prog_cache = {}


def _plan_tiles(max_load):
    """Token-tile sizes covering max_load exactly, ascending-gate order.

    Head tiles are 512 (one fp32 PSUM bank; wide enough that the L1
    strip groups don't outrun the w1 strip feed in the DMA-bound head).
    The remainder is split into two roughly equal tiles >= 128 at the
    (expensive, high-gate) tail so its L2/evictions overlap better.
    """
    C = max(int(max_load), 256)
    tiles = []
    rest = C
    while rest > 512:
        if rest - 512 >= 256 or rest - 512 == 0:
            tiles.append(512)
            rest -= 512
        else:
            a = rest // 2
            tiles.extend([rest - a, a])
            rest = 0
    if rest:
        tiles.append(rest)
    return C, tiles


def _bounds(C):
    Kx = [min(C, int(round(f * C))) for f in FRAC_X]
    Kw = [min(C, int(round(f * C))) for f in FRAC_W]
    K2 = [min(C, int(round(f * C))) for f in FRAC_2]
    Kh = [min(C, int(round(f * C))) for f in FRAC_H]
    return Kx, Kw, K2, Kh


def _build_program(C, tok_tiles, Kx, Kw, K2, Kh, n_warm=5, h_bufs=2,
                   y_bufs=3, hoist=True, head_pipe=True):
    """Per-core Bass program: one expert's 2-layer MLP over C tokens."""
    from contextlib import ExitStack

    import concourse.tile as tile
    from concourse import bacc, mybir

    f32 = mybir.dt.float32
    bf16 = mybir.dt.bfloat16
    f16 = mybir.dt.float16
    f8 = mybir.dt.float8e4
    DR = mybir.MatmulPerfMode.DoubleRow
    RELU = mybir.ActivationFunctionType.Relu
    COPY = mybir.ActivationFunctionType.Copy
    ADD = mybir.AluOpType.add
    MULT = mybir.AluOpType.mult
    SUB = mybir.AluOpType.subtract

    nc = bacc.Bacc("TRN2", target_bir_lowering=False, debug=False,
                   num_devices=NUM_EXPERTS)

    T = len(tok_tiles)
    tile_pos = [0]
    for TT in tok_tiles:
        tile_pos.append(tile_pos[-1] + TT)
    TT0 = tok_tiles[0]
    TTe = tok_tiles[-1]

    # host-packed layouts (ascending gate order, padding at the front):
    #   w1:  [8, 128, 2, 4, 2, 128]  w1[j, p, v, s, k, r] =
    #        q(32*W1[(2s+k)*128+p, j*128+r]) for v=0 (hi); residual v=1
    #   w2:  [8, 128, 2, 4, 2, 128]  same over W2 with j->o output strips
    #   x0:  [128, 4, 2, TT0]        head tile's xhi, own contiguous tensor
    #   xhi: [128, 4, 2, C]          xhi[p, s, k, c] = q(8*x_c[(2s+k)*128+p])
    #   xlo: [128, sum_s 2*Kx[s]]    per-slab suffix residuals, concatenated
    #   bb:  [128, 16]               [:, j] = 8*b1[j*128+p]; [:, 8+o] = 256*b2
    #   yT:  [128, 8, C] f16         yT[p, o, c] = y_c[o*128+p]
    #   yE:  [128, 8*TTe] f16        last tile's output (contiguous tail)
    w1_d = nc.dram_tensor("w1", [8, 128, 2, 4, 2, 128], f8,
                          kind="ExternalInput").ap()
    w2_d = nc.dram_tensor("w2", [8, 128, 2, 4, 2, 128], f8,
                          kind="ExternalInput").ap()
    x0_d = nc.dram_tensor("x0", [128, 4, 2, TT0], f8,
                          kind="ExternalInput").ap()
    xhi_d = nc.dram_tensor("xhi", [128, 4, 2, C], f8,
                           kind="ExternalInput").ap()
    # x residuals for the last Kxm token positions, all 4 slabs padded to
    # Kxm so slab slices are plain strided views (zeros where unused)
    Kxm = max(max(Kx), 1)
    xlo_d = nc.dram_tensor("xlo", [128, 4, 2, Kxm], f8,
                           kind="ExternalInput").ap()
    bb_d = nc.dram_tensor("bb", [128, 24], f32, kind="ExternalInput").ap()
    yT_d = nc.dram_tensor("yT", [128, 8, C], f16, kind="ExternalOutput").ap()
    yE_d = nc.dram_tensor("yE", [128, 8 * TTe], f16,
                          kind="ExternalOutput").ap()

    with tile.TileContext(nc) as tc, ExitStack() as ctx:
        wpool = ctx.enter_context(tc.tile_pool(name="w", bufs=1))
        cpool = ctx.enter_context(tc.tile_pool(name="const", bufs=1))
        xpool = ctx.enter_context(tc.tile_pool(name="x", bufs=1))
        hpool = ctx.enter_context(tc.tile_pool(name="h", bufs=h_bufs))
        bpool = ctx.enter_context(tc.tile_pool(name="hb", bufs=3))
        ypool = ctx.enter_context(tc.tile_pool(name="y", bufs=y_bufs))
        php = ctx.enter_context(tc.tile_pool(name="ph", bufs=4, space="PSUM"))
        pyp = ctx.enter_context(tc.tile_pool(name="py", bufs=4, space="PSUM"))

        # PE warm-up: on-chip zeros so dummy matmuls ride out the HAM clock
        # ramp while the DMA-bound head (w1 strip 0 + x0) lands.
        wz = cpool.tile([1, 640], bf16, tag="wz")
        nc.vector.memzero(wz[:])
        for _ in range(n_warm):
            warm = php.tile([128, 512], f32, tag="ph")
            nc.tensor.matmul(warm[:], wz[:, 0:128], wz[:, 128:640],
                             start=True, stop=True)

        # ---- DMA emission in consumption order ----
        # SP queue (strictly ordered): w1 strips, w2 strips, xlo, xhi
        # tiles 1..T-1, then per-tile y outputs as they are produced.
        # ACT queue: x0 (head tile) + biases, landing alongside w1 strip 0.
        w1_sb = []
        w1_0 = wpool.tile([128, 2, 4, 2, 128], f8, tag="w1_0")
        nc.sync.dma_start(w1_0[:], w1_d[0])
        w1_sb.append(w1_0)

        # x0 + biases ride the ACT queue, landing alongside w1 strip 0
        # (per-slab x0 splits and SP placement both measured slower: the
        # extra HWDGE dispatches outweigh the earlier first slab).
        x0_sb = xpool.tile([128, 4, 2, TT0], f8, tag="x0")
        bb_sb = cpool.tile([128, 24], f32, tag="bb")
        nc.scalar.dma_start(x0_sb[:], x0_d[:])
        nc.scalar.dma_start(bb_sb[:], bb_d[:])
        b1_sb = bb_sb[:, 0:8]
        b2_sb = bb_sb[:, 8:16]       # 256*b2 (DVE tensor_scalar path)
        b2r_sb = bb_sb[:, 16:24]     # raw b2 (ACT Identity path)

        x1_early = None
        pipe2 = head_pipe and T > 2 and tok_tiles[1] >= 512
        if pipe2:
            # head pipeline: x1 ships right after x0, before w1 strips
            # 1-7 — L1(0)/L1(1) strip-interleave consumes each strip for
            # both tiles (~1.4us/strip vs the 0.73us feed), so the weight
            # stream never starves the PE during the DMA-bound head.
            x1_early = xpool.tile([128, 4, 2, tok_tiles[1]], f8, tag="x1")
            nc.sync.dma_start(
                x1_early[:], xhi_d[:, :, :, tile_pos[1]:tile_pos[2]])
        # w2 strip 0 slots into the w1 stream early enough that L2(0) can
        # start right as layer 1 finishes; w1 strips 6-7 still outrun
        # their consumption by the L1 groups.
        w2_sb = [None] * 8
        for j in range(1, 8):
            if j == 6 and not pipe2:
                w2_0 = wpool.tile([128, 2, 4, 2, 128], f8, tag="w2_0")
                nc.sync.dma_start(w2_0[:], w2_d[0])
                w2_sb[0] = w2_0
            w1_j = wpool.tile([128, 2, 4, 2, 128], f8, tag=f"w1_{j}")
            nc.sync.dma_start(w1_j[:], w1_d[j])
            w1_sb.append(w1_j)
        for o in range(8):
            if w2_sb[o] is not None:
                continue
            if o == 7 and x1_early is None and T > 1 and tok_tiles[1] >= 512:
                # x1 slots in just before the last w2 strip: it lands right
                # as tile 1's L1 starts, and w2 strip 7 still beats L2(0)'s
                # final o-group
                x1_early = xpool.tile([128, 4, 2, tok_tiles[1]], f8,
                                      tag="x1")
                nc.sync.dma_start(
                    x1_early[:], xhi_d[:, :, :, tile_pos[1]:tile_pos[2]])
            w2_o = wpool.tile([128, 2, 4, 2, 128], f8, tag=f"w2_{o}")
            nc.sync.dma_start(w2_o[:], w2_d[o])
            w2_sb[o] = w2_o

        # xhi tiles: tile 0 from x0; tiles 1.. from xhi_d slices, with
        # trailing sub-512 tiles grouped into one transfer (>=512B runs).
        # The xlo bundle slots in only before the first tile that needs
        # it (suffix terms start ~70% of the way through the schedule).
        x_sb = [None] * T
        x_base = [0] * T       # column offset of tile t inside its sb tile
        x_sb[0] = x0_sb
        xlo_sb = xpool.tile([128, 4, 2, Kxm], f8, tag="xlo")
        xlo_sent = False
        t = 1
        if x1_early is not None:
            x_sb[1] = x1_early
            t = 2
        while t < T:
            if not xlo_sent and tile_pos[t + 1] > C - max(Kx):
                nc.sync.dma_start(xlo_sb[:], xlo_d[:])
                xlo_sent = True
            if tok_tiles[t] >= 512 or t == T - 1:
                xt = xpool.tile([128, 4, 2, tok_tiles[t]], f8, tag=f"x{t}")
                nc.sync.dma_start(
                    xt[:], xhi_d[:, :, :, tile_pos[t]:tile_pos[t + 1]])
                x_sb[t] = xt
                t += 1
            else:
                w = C - tile_pos[t]
                xt = xpool.tile([128, 4, 2, w], f8, tag=f"x{t}")
                nc.sync.dma_start(xt[:], xhi_d[:, :, :, tile_pos[t]:C])
                for u in range(t, T):
                    x_sb[u] = xt
                    x_base[u] = tile_pos[u] - tile_pos[t]
                t = T
                break
        if not xlo_sent:
            nc.sync.dma_start(xlo_sb[:], xlo_d[:])

        def emit_l1_strip(t, j, h_dst, alt_pool=False):
            """Layer 1 strip j for tile t -> h_hi[:, j] (+ h_lo suffix)."""
            TT = tok_tiles[t]
            pos = tile_pos[t]
            hhi_t, hlo_t = h_dst
            xs = x_sb[t]
            xb = x_base[t]
            if True:
                pool, tag = ((pyp, "py") if alt_pool and j % 2 else
                             (php, "ph"))
                ph = pool.tile([128, 512], f32, tag=tag)
                # slab 0 hi first (full width, start=True)
                nc.tensor.matmul(ph[:, 0:TT], w1_sb[j][:, 0, 0],
                                 xs[:, 0, :, xb:xb + TT],
                                 start=True, stop=False, perf_mode=DR)
                for s in range(4):
                    if s > 0 and s < 3:
                        nc.tensor.matmul(ph[:, 0:TT], w1_sb[j][:, 0, s],
                                         xs[:, s, :, xb:xb + TT],
                                         start=False, stop=False,
                                         perf_mode=DR)
                    # W1 residual (suffix)
                    v0 = max(0, (C - Kw[s]) - pos)
                    if v0 < TT and Kw[s] > 0:
                        nc.tensor.matmul(ph[:, v0:TT], w1_sb[j][:, 1, s],
                                         xs[:, s, :, xb + v0:xb + TT],
                                         start=False, stop=False,
                                         perf_mode=DR)
                    # x residual (suffix of highest-gate tokens)
                    u0 = max(0, (C - Kx[s]) - pos)
                    if u0 < TT and Kx[s] > 0:
                        i0 = pos + u0 - (C - Kxm)
                        nc.tensor.matmul(ph[:, u0:TT], w1_sb[j][:, 0, s],
                                         xlo_sb[:, s, :,
                                                i0:i0 + (TT - u0)],
                                         start=False, stop=False,
                                         perf_mode=DR)
                # slab 3 hi last (full width, stop=True)
                nc.tensor.matmul(ph[:, 0:TT], w1_sb[j][:, 0, 3],
                                 xs[:, 3, :, xb:xb + TT],
                                 start=False, stop=True, perf_mode=DR)
                # eviction: h8 = relu(psum*2^-5 + 8*b1) (= 8*h). Tokens in
                # the h-lo suffix go through a bf16 intermediate (so the
                # fp8 residual can be formed); tokens before it evict
                # straight to fp8 — exactly matching the host simulator.
                hs = max(0, (C - Kh[j // 2]) - pos)
                if hs < TT:
                    if hs > 0:
                        nc.scalar.activation(hhi_t[:, j, 0:hs],
                                             ph[:, 0:hs], RELU,
                                             bias=b1_sb[:, j:j + 1],
                                             scale=2.0 ** -5)
                    hb = bpool.tile([128, TT], bf16, tag="hb")
                    nc.scalar.activation(hb[:, 0:TT - hs], ph[:, hs:TT],
                                         RELU, bias=b1_sb[:, j:j + 1],
                                         scale=2.0 ** -5)
                    nc.scalar.activation(hhi_t[:, j, hs:TT],
                                         hb[:, 0:TT - hs], COPY)
                    nc.vector.tensor_tensor(hlo_t[:, j, hs:TT],
                                            hb[:, 0:TT - hs],
                                            hhi_t[:, j, hs:TT], op=SUB)
                else:
                    nc.scalar.activation(hhi_t[:, j, 0:TT], ph[:, 0:TT],
                                         RELU, bias=b1_sb[:, j:j + 1],
                                         scale=2.0 ** -5)

        def emit_l1(t, h_dst, alt_pool=False):
            for j in range(8):
                emit_l1_strip(t, j, h_dst, alt_pool=alt_pool)

        def emit_l2_group(t, h_src, o, py, c0, c1):
            """One L2 PSUM accumulation group: o-strip columns [c0, c1)."""
            TT = tok_tiles[t]
            pos = tile_pos[t]
            hhi_t, hlo_t = h_src
            nc.tensor.matmul(py[:, c0:c1], w2_sb[o][:, 0, 0],
                             hhi_t[:, 0:2, c0:c1],
                             start=True, stop=False, perf_mode=DR)
            for s in range(4):
                if s > 0 and s < 3:
                    nc.tensor.matmul(py[:, c0:c1], w2_sb[o][:, 0, s],
                                     hhi_t[:, 2 * s:2 * s + 2, c0:c1],
                                     start=False, stop=False, perf_mode=DR)
                v0 = max(c0, (C - K2[s]) - pos)
                if v0 < c1 and K2[s] > 0:
                    nc.tensor.matmul(py[:, v0:c1], w2_sb[o][:, 1, s],
                                     hhi_t[:, 2 * s:2 * s + 2, v0:c1],
                                     start=False, stop=False, perf_mode=DR)
                u0 = max(c0, (C - Kh[s]) - pos)
                if u0 < c1 and Kh[s] > 0:
                    nc.tensor.matmul(py[:, u0:c1], w2_sb[o][:, 0, s],
                                     hlo_t[:, 2 * s:2 * s + 2, u0:c1],
                                     start=False, stop=False, perf_mode=DR)
            nc.tensor.matmul(py[:, c0:c1], w2_sb[o][:, 0, 3],
                             hhi_t[:, 6:8, c0:c1],
                             start=False, stop=True, perf_mode=DR)

        def emit_l2(t, h_src, split_dma, tail=False):
            """Layer 2 for tile t from h_src = (hhi, hlo)."""
            TT = tok_tiles[t]
            pos = tile_pos[t]
            yt = ypool.tile([128, 8 * TT], f16, tag="y")
            for o in range(8):
                pool = pyp if (not tail or o % 2 == 0) else php
                py = pool.tile([128, 512], f32,
                               tag=("py" if pool is pyp else "ph"))
                emit_l2_group(t, h_src, o, py, 0, TT)
                # evict: y = (psum + 256*b2) * 2^-8 -> fp16 (ACT/DVE
                # alternation on the tail measured slightly slower)
                nc.vector.tensor_scalar(yt[:, o * TT:(o + 1) * TT],
                                        py[:, 0:TT], b2_sb[:, o:o + 1],
                                        2.0 ** -8, op0=ADD, op1=MULT)
                if split_dma:
                    nc.sync.dma_start(yT_d[:, o, pos:pos + TT],
                                      yt[:, o * TT:(o + 1) * TT])
                if tail and o == 3:
                    nc.sync.dma_start(yE_d[:, 0:4 * TT], yt[:, 0:4 * TT])
                if tail and o == 6:
                    # leave only a tiny (1-strip) transfer on the critical
                    # tail after the final o=7 eviction
                    nc.sync.dma_start(yE_d[:, 4 * TT:7 * TT],
                                      yt[:, 4 * TT:7 * TT])
            if not split_dma:
                if tail:
                    nc.sync.dma_start(yE_d[:, 7 * TT:], yt[:, 7 * TT:])
                else:
                    nc.sync.dma_start(yT_d[:, :, pos:pos + TT], yt[:])

        # PE section order: L1(0), L2(0), ..., with the (expensive) last
        # tile's L1 hoisted before L2(T-2) so its evictions hide under
        # matmuls, and tile T-2's output leaving per-o-strip.
        h_tiles = []
        for t in range(T):
            TT = tok_tiles[t]
            hhi_t = hpool.tile([128, 8, TT], f8, tag="hhi")
            hlo_t = hpool.tile([128, 8, TT], f8, tag="hlo")
            h_tiles.append((hhi_t, hlo_t))
        t0 = 0
        if pipe2:
            # head pipeline: strip-interleave L1(0)/L1(1) so each w1 strip
            # is consumed for both tiles while hot, then pair L2(0)/L2(1)
            # over the w2 stream the same way.
            for j in range(8):
                emit_l1_strip(0, j, h_tiles[0])
                emit_l1_strip(1, j, h_tiles[1])
            yt0 = ypool.tile([128, 8 * tok_tiles[0]], f16, tag="y")
            yt1 = ypool.tile([128, 8 * tok_tiles[1]], f16, tag="y")
            for o in range(8):
                for t, yt in ((0, yt0), (1, yt1)):
                    TT = tok_tiles[t]
                    py = pyp.tile([128, 512], f32, tag="py")
                    emit_l2_group(t, h_tiles[t], o, py, 0, TT)
                    nc.vector.tensor_scalar(yt[:, o * TT:(o + 1) * TT],
                                            py[:, 0:TT],
                                            b2_sb[:, o:o + 1],
                                            2.0 ** -8, op0=ADD, op1=MULT)
            for t, yt in ((0, yt0), (1, yt1)):
                nc.sync.dma_start(
                    yT_d[:, :, tile_pos[t]:tile_pos[t + 1]], yt[:])
            t0 = 2
        for t in range(t0, T):
            if t < T - 1:
                emit_l1(t, h_tiles[t])
                if hoist and t == T - 2:
                    emit_l1(T - 1, h_tiles[T - 1], alt_pool=True)
                emit_l2(t, h_tiles[t], split_dma=(t == T - 2))
            else:
                if not hoist or T == 1:
                    emit_l1(t, h_tiles[t])
                emit_l2(t, h_tiles[t], split_dma=False, tail=True)

    nc.compile()
    return nc


def _route(x, Wg, bg):
    """Host gating: fp32 softmax + top-2, matching jax.lax.top_k semantics."""
    logits = x @ Wg + bg
    m = logits.max(axis=1, keepdims=True)
    e = np.exp(logits - m)
    gates = e / e.sum(axis=1, keepdims=True)
    # stable argsort on negated values = ties broken by lower index (jax)
    order = np.argsort(-gates, axis=1, kind="stable")[:, :TOP_K]
    return gates, order


def _make_in_maps(x, W1, b1, W2, b2, gates, order, tok_lists, C, Kx, Kh,
                  TT0):
    import ml_dtypes
    f8 = ml_dtypes.float8_e4m3fn

    def q8(v):
        return np.ascontiguousarray(v).astype(f8)

    def deq(v):
        return v.astype(np.float32)

    def pack_w(W):
        # [1024, 1024] -> [8, 128, 2, 4, 2, 128] hi/lo strips
        Ws = W * 32.0
        # Wt[o/j, p, s, k, r] = Ws[(2s+k)*128+p, j*128+r]
        Wt = Ws.reshape(4, 2, 128, 8, 128).transpose(3, 2, 0, 1, 4)
        hi = Wt.astype(f8)
        lo = (Wt - deq(hi)).astype(f8)
        return np.ascontiguousarray(
            np.stack([hi, lo], axis=2))  # [8, 128, 2, 4, 2, 128]

    Kxm = max(max(Kx), 1)
    in_maps = []
    for e in range(NUM_EXPERTS):
        toks = tok_lists[e]
        ne = len(toks)
        # ascending gate sort, padding (zeros) in FRONT
        g = gates[toks, e]
        asc = toks[np.argsort(g, kind="stable")]
        xs = np.zeros((C, D), dtype=np.float32)
        xs[C - ne:] = x[asc]
        # xhi[p, s, k, c] = q(8*xs[c, (2s+k)*128+p])
        x8 = (xs * 8.0).reshape(C, 4, 2, 128)      # [c, s, k, p]
        xhi = x8.astype(f8)                        # quantize
        xhi_t = np.ascontiguousarray(xhi.transpose(3, 1, 2, 0))
        xres = x8 - deq(xhi)                       # [c, s, k, p]
        xlo = np.zeros((128, 4, 2, Kxm), dtype=f8)
        for s in range(4):
            k = Kx[s]
            if k:
                # [p, 2, c] from residual rows of the last k tokens
                blk = xres[C - k:, s].transpose(2, 1, 0)
                xlo[:, s, :, Kxm - k:] = q8(blk)
        bb = np.concatenate([8.0 * b1[e].reshape(8, 128).T,
                             256.0 * b2[e].reshape(8, 128).T,
                             b2[e].reshape(8, 128).T], axis=1)
        in_maps.append({
            "w1": pack_w(W1[e]),
            "w2": pack_w(W2[e]),
            "x0": np.ascontiguousarray(xhi_t[:, :, :, 0:TT0]),
            "xhi": xhi_t,
            "xlo": xlo,
            "bb": np.ascontiguousarray(bb.astype(np.float32)),
        })
    return in_maps, [np.argsort(gates[tok_lists[e], e], kind="stable")
                     for e in range(NUM_EXPERTS)]


def kernel(x, W1, b1, W2, b2, Wg, bg):
    from concourse import bass_utils

    x = np.ascontiguousarray(np.asarray(x, dtype=np.float32))
    W1 = np.asarray(W1, dtype=np.float32)
    b1 = np.asarray(b1, dtype=np.float32)
    W2 = np.asarray(W2, dtype=np.float32)
    b2 = np.asarray(b2, dtype=np.float32)
    Wg = np.asarray(Wg, dtype=np.float32)
    bg = np.asarray(bg, dtype=np.float32)
    n = x.shape[0]

    gates, order = _route(x, Wg, bg)
    tok_lists = [np.where((order == e).any(axis=1))[0]
                 for e in range(NUM_EXPERTS)]
    max_load = max(len(t) for t in tok_lists)
    C, tok_tiles = _plan_tiles(max_load)
    Kx, Kw, K2, Kh = _bounds(C)

    key = (C, tuple(tok_tiles), tuple(Kx), tuple(Kw), tuple(K2), tuple(Kh))
    if key not in _prog_cache:
        _prog_cache[key] = _build_program(C, tok_tiles, Kx, Kw, K2, Kh)
    nc = _prog_cache[key]

    in_maps, asc_orders = _make_in_maps(
        x, W1, b1, W2, b2, gates, order, tok_lists, C, Kx, Kh, tok_tiles[0])
    res = bass_utils.run_bass_kernel_spmd(nc, in_maps,
                                          list(range(NUM_EXPERTS)))

    TTe = tok_tiles[-1]
    out = np.zeros((n, D), dtype=np.float32)
    for e in range(NUM_EXPERTS):
        toks = tok_lists[e]
        ne = len(toks)
        yT = np.asarray(res.results[e]["yT"], dtype=np.float32)
        yE = np.asarray(res.results[e]["yE"], dtype=np.float32)
        yT[:, :, C - TTe:] = yE.reshape(128, 8, TTe)
        # yT[p, o, c] -> y[c, o*128+p]; positions C-ne.. hold the sorted toks
        y = yT[:, :, C - ne:].transpose(2, 1, 0).reshape(ne, D)
        asc = toks[asc_orders[e]]
        out[asc] += gates[asc, e][:, None] * y
    return out
